# revision 11
# baseline (speedup 1.0000x reference)
"""BSCE loss with adaptive gamma — Trainium2 Bass kernel, 8-core data parallel.

Math (per row n of x[N=65536, C=1000], t = target[n]):
    s       = sum_c exp(x[n, c])           (randn inputs -> no max-sub needed)
    xt      = x[n, t]
    nlp     = ln(s) - xt                   (= -log softmax prob of true class)
    p       = exp(xt)/s = exp(-nlp)
    gamma   = 5 if p < 0.2 else 3
    sum_c |onehot - softmax| == 2*(1-p)    (exact identity)
    loss    = sum_n (2-2p)^gamma * nlp

v2 design (fp16 stream, two-engine exp), measured per-seg costs in ns:
 - x cast to fp16 on host: DMA 16.4 MB/core (~38 us busy) vs 100 us for f32.
 - Per chunk of w rows: `a` rows get exact ACT exp with accum_out row sums
   (1127+279); the rest get approximate exp on DVE: tensor_scalar computes
   round(1477.32*x + 15302) into int16 (420), whose bits read as fp16 ARE
   2^(x*log2e) with mean-zero sawtooth error <= 3.8% (Schraudolph; measured).
   Their row sums via fp16 fold chains + one tensor_reduce (~720/row).
 - xt gather: rows host-sorted by target; slot q's target lies in a static
   64-wide window at LO_chunk + 16k.  The host ships a one-hot window mask;
   the gather is one fp16 multiply over a strided window view + one
   tensor_reduce per chunk (~1 us/chunk vs ~330 ns/slot for per-slot STTs).
   Slot 0 takes the sort overflow; a full-width STT handles it.
 - Tail: ln(s) via inverse bit trick on DVE (no ACT Ln -> one table load),
   p = ACT exp(-nlp), gamma select + powers + final sum on DVE.

Fallback (windows violated or |x| > 11, int16 trick unsafe): the original
all-f32 all-ACT-exp kernel below, which is input-agnostic.
"""

import numpy as np

N_FULL, C = 65536, 1000
NCORES = 8
NS = N_FULL // NCORES   # 8192 rows per core
P = 128
R = NS // P             # 64 row slots per partition

CHUNKS = [2, 6, 8, 8, 8, 8, 8, 8, 6, 2]     # rows/partition per chunk
A_SEGS = [1, 2, 3, 4, 3, 4, 3, 4, 2, 1]     # ACT exact-exp rows per chunk
assert sum(CHUNKS) == R and all(a <= w for a, w in zip(A_SEGS, CHUNKS))
STT_W = 64                    # gather window width (targets sorted; ~6 sigma)
WSTEP = 16                    # window advance per slot within a chunk

LOG2E = 1.4426950408889634
A_SCH = 1024.0 * LOG2E
B_SCH = 15302.0               # 15360 - 1024*0.0566 (mean-zero sawtooth)
LN2 = 0.6931471805599453
S1_LN = LN2 / (1 << 23)       # schraudolph-inverse for ln(s), s f32
S2_LN = -127.0 * LN2
K_LN = 0.349 * LN2            # quadratic mantissa correction k*m*(1-m)
X_ABS_LIMIT = 11.0            # |x| beyond this -> int16 trick unsafe


def _chunk_lo():
    """Window base per chunk: slot col+k scans [lo + 16k, lo + 16k + 64)."""
    los = []
    col = 0
    for j, w in enumerate(CHUNKS):
        if col == 0:
            lo = C - STT_W  # overflow slot: the 128 largest targets
        else:
            lo = int(round(15.625 * col - 39.8 + 0.1875 * (w - 1)))
            lo = max(0, min(C - STT_W - WSTEP * (w - 1), lo))
        los.append(lo)
        col += w
    return los

CHUNK_LO = _chunk_lo()
# chunk 0: slot 0 (sort overflow = largest targets) scans [936, 1000);
# slot 1 (smallest targets) scans [0, 64).  AP stride C + step with
# lo0=936, step=-936+... encoded as LO=936, STEP=C-936+...: offset_k =
# 936 + (C + (-936 + 64 - 64))*k ... simpler: explicit per-chunk pairs.
CHUNK_STEP = [-936] + [WSTEP] * (len(CHUNKS) - 1)

_built = {}


def _build_fast():
    if "fast" in _built:
        return _built["fast"]
    from concourse import bacc, mybir
    from concourse.ap import AP
    from concourse.tile import TileContext

    f32 = mybir.dt.float32
    f16 = mybir.dt.float16
    i16 = mybir.dt.int16
    i32 = mybir.dt.int32
    Alu = mybir.AluOpType
    Act = mybir.ActivationFunctionType

    nc = bacc.Bacc()
    x = nc.declare_dram_parameter("x", [NS, C], f16, isOutput=False)
    msk = nc.declare_dram_parameter("wmask", [P, R * STT_W], f16, isOutput=False)
    out = nc.declare_dram_parameter("out", [P, 1], f32, isOutput=True)

    with TileContext(nc) as tc:
        with (
            tc.tile_pool(name="const", bufs=1) as cpool,
            tc.tile_pool(name="xp", bufs=6) as xpool,
            tc.tile_pool(name="ep", bufs=3) as epool,
            tc.tile_pool(name="fp", bufs=2) as fpool,
            tc.tile_pool(name="st", bufs=1) as stp,
        ):
            mask = cpool.tile([P, R * STT_W], f16)
            s_act = stp.tile([P, R], f32)
            s_dve = stp.tile([P, R], f32)
            xt_all = stp.tile([P, R], f32)
            nc.gpsimd.memset(s_act[:], 0.0)
            nc.vector.memset(s_dve[:], 0.0)

            xv = x[:].rearrange("(p q) c -> p (q c)", p=P)  # [128, 64000]
            # chunk 0's x leads the qSP ring so ACT starts ASAP; the 1 MB
            # mask follows (first needed by chunk 0's gather, ~6 us in).
            xtiles = [
                xpool.tile([P, w * C], f16, tag="x", name=f"xt{j}")
                for j, w in enumerate(CHUNKS)
            ]
            nc.sync.dma_start(out=xtiles[0][:], in_=xv[:, : CHUNKS[0] * C])
            c1 = CHUNKS[0]
            nc.sync.dma_start(
                out=xtiles[1][:], in_=xv[:, c1 * C : (c1 + CHUNKS[1]) * C]
            )
            nc.sync.dma_start(out=mask[:], in_=msk[:])
            col = 0
            pending = []
            for j, w in enumerate(CHUNKS):
                a = A_SEGS[j]
                nd = w - a
                xt_tile = xtiles[j]
                if j > 1:
                    nc.sync.dma_start(
                        out=xt_tile[:], in_=xv[:, col * C : (col + w) * C]
                    )
                esca = epool.tile([P, a * C], f16, tag="esca")
                esc = epool.tile([P, nd * C if nd else C], f16, tag="esc")
                # ACT: exact exp + accum row sums for the first a rows
                for q in range(a):
                    cq = col + q
                    nc.scalar.activation(
                        esca[:, q * C : (q + 1) * C],
                        xt_tile[:, q * C : (q + 1) * C],
                        Act.Exp,
                        accum_out=s_act[:, cq : cq + 1],
                    )
                # DVE: schraudolph exp -> int16 bits == fp16 exp approx
                if nd:
                    nc.vector.tensor_scalar(
                        esc[:, : nd * C].bitcast(i16),
                        xt_tile[:, a * C : w * C],
                        A_SCH,
                        B_SCH,
                        Alu.mult,
                        Alu.add,
                    )
                # row sums of the schraudolph rows: fold 1000->500->250 via
                # in-place SBUF->SBUF accumulate-DMA (SWDGE CCE ADD; the DMA
                # engines are ~40% idle), then one DVE reduce of [nd, 250].
                # The reduce is issued one chunk late so DVE never waits on
                # the DMA completion.
                if nd:
                    ev = esc[:, : nd * C].rearrange(
                        "p (q c) -> p q c", q=nd
                    )
                    h1, h2 = C // 2, C // 4
                    nc.gpsimd.dma_start(
                        out=ev[:, :, :h1], in_=ev[:, :, h1:], accum_op=Alu.add
                    )
                    nc.gpsimd.dma_start(
                        out=ev[:, :, :h2],
                        in_=ev[:, :, h2:h1],
                        accum_op=Alu.add,
                    )
                    pending.append(
                        (s_dve[:, col + a : col + w], ev[:, :, :h2])
                    )
                if len(pending) > 1:
                    sl, rv = pending.pop(0)
                    nc.vector.tensor_reduce(
                        sl, rv, axis=mybir.AxisListType.X, op=Alu.add
                    )
                # DVE: windowed mask gather for all w slots of this chunk
                xw = xt_tile[:]
                win = AP(
                    tensor=xw.tensor,
                    offset=xw.offset + CHUNK_LO[j],
                    ap=[
                        [int(xw.ap[0][0]), int(xw.ap[0][1])],
                        [C + CHUNK_STEP[j], w],
                        [1, STT_W],
                    ],
                )
                prod = fpool.tile([P, w * STT_W], f16, tag="pr")
                pv = prod[:].rearrange("p (q i) -> p q i", q=w)
                mv = mask[:, col * STT_W : (col + w) * STT_W].rearrange(
                    "p (q i) -> p q i", q=w
                )
                nc.vector.tensor_tensor(pv, mv, win, Alu.mult)
                nc.vector.tensor_reduce(
                    xt_all[:, col : col + w],
                    pv,
                    axis=mybir.AxisListType.X,
                    op=Alu.add,
                )
                col += w
            for sl, rv in pending:
                nc.vector.tensor_reduce(
                    sl, rv, axis=mybir.AxisListType.X, op=Alu.add
                )

            # ---- tail ----
            lns = stp.tile([P, R], f32)
            nlp = stp.tile([P, R], f32)
            pv_t = stp.tile([P, R], f32)
            base = stp.tile([P, R], f32)
            b2 = stp.tile([P, R], f32)
            b3 = stp.tile([P, R], f32)
            m = stp.tile([P, R], f32)
            me = stp.tile([P, R], f32)
            diff = stp.tile([P, R], f32)
            term = stp.tile([P, R], f32)
            mi = stp.tile([P, R], i32)
            mf = stp.tile([P, R], f32)
            w1 = stp.tile([P, R], f32)
            corr = stp.tile([P, R], f32)
            lin = stp.tile([P, R], f32)

            s_all = stp.tile([P, R], f32)
            nc.vector.tensor_tensor(s_all[:], s_act[:], s_dve[:], Alu.add)
            # ln(s) = (e + m)*ln2 - 127*ln2 + k*ln2*m*(1-m), m = mantissa
            nc.vector.tensor_scalar(
                mi[:], s_all[:].bitcast(i32), 0x007FFFFF, None, Alu.bitwise_and
            )
            nc.vector.tensor_scalar(
                mf[:], mi[:], 2.0 ** -23, 0.0, Alu.mult, Alu.add
            )
            nc.vector.tensor_scalar(
                w1[:], mf[:], -1.0, 1.0, Alu.mult, Alu.add
            )
            nc.vector.tensor_tensor(corr[:], mf[:], w1[:], Alu.mult)
            nc.vector.tensor_scalar(
                lin[:], s_all[:].bitcast(i32), S1_LN, S2_LN, Alu.mult, Alu.add
            )
            nc.vector.scalar_tensor_tensor(
                lns[:], corr[:], K_LN, lin[:], Alu.mult, Alu.add
            )
            nc.vector.tensor_tensor(nlp[:], lns[:], xt_all[:], Alu.subtract)
            nc.scalar.activation(pv_t[:], nlp[:], Act.Exp, scale=-1.0)
            nc.vector.tensor_scalar(
                base[:], pv_t[:], -2.0, 2.0, Alu.mult, Alu.add
            )
            nc.vector.tensor_tensor(b2[:], base[:], base[:], Alu.mult)
            nc.vector.tensor_tensor(b3[:], b2[:], base[:], Alu.mult)
            nc.vector.tensor_scalar(m[:], pv_t[:], 0.2, None, Alu.is_lt)
            nc.vector.scalar_tensor_tensor(
                me[:], b2[:], -1.0, m[:], Alu.add, Alu.mult
            )
            nc.vector.scalar_tensor_tensor(
                diff[:], me[:], 1.0, b3[:], Alu.add, Alu.mult
            )
            nc.vector.tensor_tensor(term[:], diff[:], nlp[:], Alu.mult)
            osb = stp.tile([P, 1], f32)
            nc.vector.tensor_reduce(
                osb[:], term[:], axis=mybir.AxisListType.X, op=Alu.add
            )
            nc.sync.dma_start(out=out[:], in_=osb[:])

    nc.finalize()
    _built["fast"] = nc
    return nc


# ---------------------------------------------------------------------------
# Fallback: original all-f32 kernel (full-width scans, exact ACT exp).
# Correct for any input values / target distribution.
# ---------------------------------------------------------------------------

FB_CHUNKS = [1, 2, 5, 8, 8, 8, 8, 8, 8, 4, 2, 1, 1]
FB_WMAX = max(FB_CHUNKS) * C
FB_ACC = {3: 8, 7: 8, 8: 2, 9: 4, 10: 2}


def _build_fallback():
    if "fb" in _built:
        return _built["fb"]
    from concourse import bacc, mybir
    from concourse.tile import TileContext

    f32 = mybir.dt.float32
    f16 = mybir.dt.float16
    Alu = mybir.AluOpType
    Act = mybir.ActivationFunctionType

    nc = bacc.Bacc()
    x = nc.declare_dram_parameter("x", [NS, C], f32, isOutput=False)
    tgt = nc.declare_dram_parameter("tgt", [P, R], f32, isOutput=False)
    iot = nc.declare_dram_parameter("iota", [P, C], f32, isOutput=False)
    out = nc.declare_dram_parameter("out", [P, 1], f32, isOutput=True)

    with TileContext(nc) as tc:
        with (
            tc.tile_pool(name="const", bufs=1) as cpool,
            tc.tile_pool(name="xp", bufs=4) as xpool,
            tc.tile_pool(name="ep", bufs=2) as epool,
            tc.tile_pool(name="st", bufs=1) as stp,
        ):
            iota = cpool.tile([P, C], f32)
            nc.sync.dma_start(out=iota[:], in_=iot[:])
            tgt_sb = cpool.tile([P, R], f32)
            nc.sync.dma_start(out=tgt_sb[:], in_=tgt[:])

            s_all = stp.tile([P, R], f32)
            xt_all = stp.tile([P, R], f32)
            gsc_full = stp.tile([P, C], f32)

            ext = stp.tile([P, R], f32)
            lse = stp.tile([P, R], f32)
            rs = stp.tile([P, R], f32)
            pv = stp.tile([P, R], f32)
            nlp = stp.tile([P, R], f32)
            base = stp.tile([P, R], f32)
            b2 = stp.tile([P, R], f32)
            b3 = stp.tile([P, R], f32)
            m = stp.tile([P, R], f32)
            me = stp.tile([P, R], f32)
            diff = stp.tile([P, R], f32)
            term = stp.tile([P, R], f32)

            xv = x[:].rearrange("(p q) c -> p (q c)", p=P)
            col = 0
            for j, w in enumerate(FB_CHUNKS):
                xt_tile = xpool.tile([P, FB_WMAX], f32, tag="x")
                nc.sync.dma_start(
                    out=xt_tile[:, : w * C], in_=xv[:, col * C : (col + w) * C]
                )
                nacc = FB_ACC.get(j, 0)
                nbig = w - nacc
                esc = epool.tile([P, FB_WMAX], f16, tag="esc")
                if nbig:
                    nc.scalar.activation(
                        esc[:, : nbig * C], xt_tile[:, : nbig * C], Act.Exp
                    )
                for q in range(nbig, w):
                    cq = col + q
                    nc.scalar.activation(
                        esc[:, q * C : (q + 1) * C],
                        xt_tile[:, q * C : (q + 1) * C],
                        Act.Exp,
                        accum_out=s_all[:, cq : cq + 1],
                    )
                for q in range(w):
                    cq = col + q
                    nc.vector.scalar_tensor_tensor(
                        gsc_full[:],
                        iota[:],
                        tgt_sb[:, cq : cq + 1],
                        xt_tile[:, q * C : (q + 1) * C],
                        Alu.is_equal,
                        Alu.mult,
                        accum_out=xt_all[:, cq : cq + 1],
                    )
                if nbig:
                    nc.vector.tensor_reduce(
                        s_all[:, col : col + nbig],
                        esc[:, : nbig * C].rearrange("p (q c) -> p q c", q=nbig),
                        axis=mybir.AxisListType.X,
                        op=Alu.add,
                    )
                col += w

            nc.scalar.activation(ext[:], xt_all[:], Act.Exp)
            nc.scalar.activation(lse[:], s_all[:], Act.Ln)
            nc.vector.reciprocal(rs[:], s_all[:])
            nc.vector.tensor_tensor(pv[:], ext[:], rs[:], Alu.mult)
            nc.vector.tensor_tensor(nlp[:], lse[:], xt_all[:], Alu.subtract)
            nc.vector.tensor_scalar(
                base[:], pv[:], -2.0, 2.0, Alu.mult, Alu.add
            )
            nc.vector.tensor_tensor(b2[:], base[:], base[:], Alu.mult)
            nc.vector.tensor_tensor(b3[:], b2[:], base[:], Alu.mult)
            nc.vector.tensor_scalar(m[:], pv[:], 0.2, None, Alu.is_lt)
            nc.vector.scalar_tensor_tensor(
                me[:], b2[:], -1.0, m[:], Alu.add, Alu.mult
            )
            nc.vector.scalar_tensor_tensor(
                diff[:], me[:], 1.0, b3[:], Alu.add, Alu.mult
            )
            nc.vector.tensor_tensor(term[:], diff[:], nlp[:], Alu.mult)
            osb = stp.tile([P, 1], f32)
            nc.vector.tensor_reduce(
                osb[:], term[:], axis=mybir.AxisListType.X, op=Alu.add
            )
            nc.sync.dma_start(out=out[:], in_=osb[:])

    nc.finalize()
    _built["fb"] = nc
    return nc


def _build(full_scan=False):
    return _build_fallback() if full_scan else _build_fast()


def _prepare_in_maps(x, target):
    """Per core: sort rows by target, assign rank r -> (p=r%128, slot q=1+r//128)
    for r < 8064; top 128 ranks -> overflow slot q=0.  Build window masks and
    verify every slot's targets sit inside its static window."""
    x = np.asarray(x)
    if x.dtype != np.float32:
        x = x.astype(np.float32)
    t = np.asarray(target).astype(np.int64)
    fallback = bool(np.max(np.abs(x)) > X_ABS_LIMIT)
    cores = []
    for cid in range(NCORES):
        tc = t[cid * NS : (cid + 1) * NS]
        order = np.argsort(tc, kind="stable")
        ranks_main = order[: 128 * (R - 1)].reshape(R - 1, P)
        perm_pq = np.empty((P, R), dtype=np.int64)
        perm_pq[:, 1:] = ranks_main.T
        perm_pq[:, 0] = order[128 * (R - 1) :]
        perm = perm_pq.reshape(-1)
        tw = tc[perm_pq]  # [P, R] target at each slot
        wmask = np.zeros((P, R, STT_W), dtype=np.float16)
        col = 0
        for j, w in enumerate(CHUNKS):
            for k in range(w):
                q = col + k
                lo = CHUNK_LO[j] + CHUNK_STEP[j] * k
                pos = tw[:, q] - lo
                if not ((pos >= 0) & (pos < STT_W)).all():
                    fallback = True
                    continue
                wmask[np.arange(P), q, pos] = 1.0
            col += w
        cores.append((perm, perm_pq, wmask))
    np_dt = np.float32 if fallback else np.float16
    iota = np.ascontiguousarray(
        np.broadcast_to(np.arange(C, dtype=np_dt), (P, C))
    )
    in_maps = []
    for cid in range(NCORES):
        perm, perm_pq, wmask = cores[cid]
        xs = np.ascontiguousarray(x[cid * NS : (cid + 1) * NS][perm].astype(np_dt))
        tg = np.ascontiguousarray(
            t[cid * NS : (cid + 1) * NS][perm_pq].astype(np_dt)
        )
        if fallback:
            im = {"x": xs, "tgt": tg, "iota": iota}
        else:
            im = {"x": xs, "wmask": np.ascontiguousarray(wmask.reshape(P, R * STT_W))}
        in_maps.append(im)
    return in_maps, fallback


def _finish(results):
    total = 0.0
    for r in results:
        total += float(np.sum(r["out"].astype(np.float64)))
    return np.asarray(total, dtype=np.float32)


def kernel(x, target):
    from concourse.bass_utils import run_bass_kernel_spmd

    in_maps, fallback = _prepare_in_maps(x, target)
    nc = _build(fallback)
    res = run_bass_kernel_spmd(nc, in_maps, core_ids=list(range(NCORES)))
    return _finish(res.results)


# revision 13
# speedup vs baseline: 2.0336x; 2.0336x over previous
"""BSCE loss with adaptive gamma — Trainium2 Bass kernel, 8-core data parallel.

Math (per row n of x[N=65536, C=1000], t = target[n]):
    s       = sum_c exp(x[n, c])           (randn inputs -> no max-sub needed)
    xt      = x[n, t]
    nlp     = ln(s) - xt                   (= -log softmax prob of true class)
    p       = exp(xt)/s = exp(-nlp)
    gamma   = 5 if p < 0.2 else 3
    sum_c |onehot - softmax| == 2*(1-p)    (exact identity)
    loss    = sum_n (2-2p)^gamma * nlp

v2 design (fp16 stream, two-engine exp), measured per-seg costs in ns:
 - x cast to fp16 on host: DMA 16.4 MB/core (~38 us busy) vs 100 us for f32.
 - Per chunk of w rows: `a` rows get exact ACT exp with accum_out row sums
   (1127+279); the rest get approximate exp on DVE: tensor_scalar computes
   round(1477.32*x + 15302) into int16 (420), whose bits read as fp16 ARE
   2^(x*log2e) with mean-zero sawtooth error <= 3.8% (Schraudolph; measured).
   Their row sums via fp16 fold chains + one tensor_reduce (~720/row).
 - xt gather: rows host-sorted by target; slot q's target lies in a static
   64-wide window at LO_chunk + 16k.  The host ships a one-hot window mask;
   the gather is one fp16 multiply over a strided window view + one
   tensor_reduce per chunk (~1 us/chunk vs ~330 ns/slot for per-slot STTs).
   Slot 0 takes the sort overflow (the 128 LARGEST targets per core), so its
   window is statically [936, 1000) - host-validated like every other slot.
 - Tail: ln(s) = (e+m)*ln2 + k*ln2*m*(1-m) from the f32 bits of s on DVE
   (max err 6e-3, no ACT Ln -> single ACT table load), p = ACT exp(-nlp),
   gamma select + powers + final sum on DVE.

Fallback (windows violated or |x| > 11, int16 trick unsafe): the original
all-f32 all-ACT-exp kernel below, which is input-agnostic.
"""

import numpy as np

N_FULL, C = 65536, 1000
NCORES = 8
NS = N_FULL // NCORES   # 8192 rows per core
P = 128
R = NS // P             # 64 row slots per partition

CHUNKS = [2, 6, 8, 8, 8, 8, 8, 8, 6, 2]     # rows/partition per chunk
A_SEGS = [1, 3, 4, 4, 4, 5, 4, 5, 3, 1]     # ACT exact-exp rows per chunk
assert sum(CHUNKS) == R and all(a <= w for a, w in zip(A_SEGS, CHUNKS))
STT_W = 64                    # gather window width (targets sorted; ~6 sigma)
WSTEP = 16                    # window advance per slot within a chunk

LOG2E = 1.4426950408889634
A_SCH = 1024.0 * LOG2E
B_SCH = 15302.0               # 15360 - 1024*0.0566 (mean-zero sawtooth)
LN2 = 0.6931471805599453
S1_LN = LN2 / (1 << 23)       # schraudolph-inverse for ln(s), s f32
S2_LN = -127.0 * LN2
K_LN = 0.349 * LN2            # quadratic mantissa correction k*m*(1-m)
X_ABS_LIMIT = 11.0            # |x| beyond this -> int16 trick unsafe


def _chunk_lo():
    """Window base per chunk: slot col+k scans [lo + 16k, lo + 16k + 64)."""
    los = []
    col = 0
    for j, w in enumerate(CHUNKS):
        if col == 0:
            lo = C - STT_W  # overflow slot: the 128 largest targets
        else:
            lo = int(round(15.625 * col - 39.8 + 0.1875 * (w - 1)))
            lo = max(0, min(C - STT_W - WSTEP * (w - 1), lo))
        los.append(lo)
        col += w
    return los

CHUNK_LO = _chunk_lo()
# chunk 0: slot 0 (sort overflow = largest targets) scans [936, 1000) of
# row 0; slot 1 (smallest targets) scans [0, 64) of row 1.  The device AP
# stride is C + STEP, so STEP=-936 makes slot k=1 land at element 1000+0.
CHUNK_STEP = [-936] + [WSTEP] * (len(CHUNKS) - 1)

_built = {}


def _build_fast():
    if "fast" in _built:
        return _built["fast"]
    from concourse import bacc, mybir
    from concourse.ap import AP
    from concourse.tile import TileContext

    f32 = mybir.dt.float32
    f16 = mybir.dt.float16
    i16 = mybir.dt.int16
    i32 = mybir.dt.int32
    Alu = mybir.AluOpType
    Act = mybir.ActivationFunctionType

    nc = bacc.Bacc()
    x = nc.declare_dram_parameter("x", [NS, C], f16, isOutput=False)
    msk = nc.declare_dram_parameter("wmask", [P, R * STT_W], f16, isOutput=False)
    out = nc.declare_dram_parameter("out", [P, 1], f32, isOutput=True)

    with TileContext(nc) as tc:
        with (
            tc.tile_pool(name="const", bufs=1) as cpool,
            tc.tile_pool(name="xp", bufs=6) as xpool,
            tc.tile_pool(name="ep", bufs=3) as epool,
            tc.tile_pool(name="fp", bufs=2) as fpool,
            tc.tile_pool(name="st", bufs=1) as stp,
        ):
            mask = cpool.tile([P, R * STT_W], f16)
            s_act = stp.tile([P, R], f32)
            s_dve = stp.tile([P, R], f32)
            xt_all = stp.tile([P, R], f32)
            nc.gpsimd.memset(s_act[:], 0.0)
            nc.vector.memset(s_dve[:], 0.0)

            xv = x[:].rearrange("(p q) c -> p (q c)", p=P)  # [128, 64000]
            # chunk 0's x leads the qSP ring so ACT starts ASAP; the 1 MB
            # mask follows (first needed by chunk 0's gather, ~6 us in).
            xtiles = [
                xpool.tile([P, w * C], f16, tag="x", name=f"xt{j}")
                for j, w in enumerate(CHUNKS)
            ]
            nc.sync.dma_start(out=xtiles[0][:], in_=xv[:, : CHUNKS[0] * C])
            c1 = CHUNKS[0]
            nc.sync.dma_start(
                out=xtiles[1][:], in_=xv[:, c1 * C : (c1 + CHUNKS[1]) * C]
            )
            nc.sync.dma_start(out=mask[:], in_=msk[:])
            col = 0
            for j, w in enumerate(CHUNKS):
                a = A_SEGS[j]
                nd = w - a
                xt_tile = xtiles[j]
                if j > 1:
                    nc.sync.dma_start(
                        out=xt_tile[:], in_=xv[:, col * C : (col + w) * C]
                    )
                esca = epool.tile([P, a * C], f16, tag="esca")
                esc = epool.tile([P, nd * C if nd else C], f16, tag="esc")
                # ACT: exact exp + accum row sums for the first a rows
                for q in range(a):
                    cq = col + q
                    nc.scalar.activation(
                        esca[:, q * C : (q + 1) * C],
                        xt_tile[:, q * C : (q + 1) * C],
                        Act.Exp,
                        accum_out=s_act[:, cq : cq + 1],
                    )
                # DVE: schraudolph exp -> int16 bits == fp16 exp approx
                if nd:
                    nc.vector.tensor_scalar(
                        esc[:, : nd * C].bitcast(i16),
                        xt_tile[:, a * C : w * C],
                        A_SCH,
                        B_SCH,
                        Alu.mult,
                        Alu.add,
                    )
                # DVE: fold-chain row sums for the schraudolph rows
                if nd:
                    ev = esc[:, : nd * C].rearrange(
                        "p (q c) -> p q c", q=nd
                    )
                    h1, h2, h3 = C // 2, C // 4, C // 8
                    fold1 = fpool.tile([P, nd * h1], f16, tag="f1")
                    f1v = fold1[:].rearrange("p (q c) -> p q c", q=nd)
                    nc.vector.tensor_tensor(
                        f1v, ev[:, :, :h1], ev[:, :, h1:], Alu.add
                    )
                    fold2 = fpool.tile([P, nd * h2], f16, tag="f2")
                    f2v = fold2[:].rearrange("p (q c) -> p q c", q=nd)
                    nc.vector.tensor_tensor(
                        f2v, f1v[:, :, :h2], f1v[:, :, h2:], Alu.add
                    )
                    fold3 = fpool.tile([P, nd * h3], f16, tag="f3")
                    f3v = fold3[:].rearrange("p (q c) -> p q c", q=nd)
                    nc.vector.tensor_tensor(
                        f3v, f2v[:, :, :h3], f2v[:, :, h3:], Alu.add
                    )
                    nc.vector.tensor_reduce(
                        s_dve[:, col + a : col + w],
                        f3v,
                        axis=mybir.AxisListType.X,
                        op=Alu.add,
                    )
                # DVE: windowed mask gather for all w slots of this chunk
                xw = xt_tile[:]
                win = AP(
                    tensor=xw.tensor,
                    offset=xw.offset + CHUNK_LO[j],
                    ap=[
                        [int(xw.ap[0][0]), int(xw.ap[0][1])],
                        [C + CHUNK_STEP[j], w],
                        [1, STT_W],
                    ],
                )
                prod = fpool.tile([P, w * STT_W], f16, tag="pr")
                pv = prod[:].rearrange("p (q i) -> p q i", q=w)
                mv = mask[:, col * STT_W : (col + w) * STT_W].rearrange(
                    "p (q i) -> p q i", q=w
                )
                nc.vector.tensor_tensor(pv, mv, win, Alu.mult)
                nc.vector.tensor_reduce(
                    xt_all[:, col : col + w],
                    pv,
                    axis=mybir.AxisListType.X,
                    op=Alu.add,
                )
                col += w

            # ---- tail ----
            lns = stp.tile([P, R], f32)
            nlp = stp.tile([P, R], f32)
            pv_t = stp.tile([P, R], f32)
            base = stp.tile([P, R], f32)
            b2 = stp.tile([P, R], f32)
            b3 = stp.tile([P, R], f32)
            m = stp.tile([P, R], f32)
            me = stp.tile([P, R], f32)
            diff = stp.tile([P, R], f32)
            term = stp.tile([P, R], f32)
            mi = stp.tile([P, R], i32)
            mf = stp.tile([P, R], f32)
            w1 = stp.tile([P, R], f32)
            corr = stp.tile([P, R], f32)
            lin = stp.tile([P, R], f32)

            s_all = stp.tile([P, R], f32)
            nc.vector.tensor_tensor(s_all[:], s_act[:], s_dve[:], Alu.add)
            # ln(s) = (e + m)*ln2 - 127*ln2 + k*ln2*m*(1-m), m = mantissa
            nc.vector.tensor_scalar(
                mi[:], s_all[:].bitcast(i32), 0x007FFFFF, None, Alu.bitwise_and
            )
            nc.vector.tensor_scalar(
                mf[:], mi[:], 2.0 ** -23, 0.0, Alu.mult, Alu.add
            )
            nc.vector.tensor_scalar(
                w1[:], mf[:], -1.0, 1.0, Alu.mult, Alu.add
            )
            nc.vector.tensor_tensor(corr[:], mf[:], w1[:], Alu.mult)
            nc.vector.tensor_scalar(
                lin[:], s_all[:].bitcast(i32), S1_LN, S2_LN, Alu.mult, Alu.add
            )
            nc.vector.scalar_tensor_tensor(
                lns[:], corr[:], K_LN, lin[:], Alu.mult, Alu.add
            )
            nc.vector.tensor_tensor(nlp[:], lns[:], xt_all[:], Alu.subtract)
            nc.scalar.activation(pv_t[:], nlp[:], Act.Exp, scale=-1.0)
            nc.vector.tensor_scalar(
                base[:], pv_t[:], -2.0, 2.0, Alu.mult, Alu.add
            )
            nc.vector.tensor_tensor(b2[:], base[:], base[:], Alu.mult)
            nc.vector.tensor_tensor(b3[:], b2[:], base[:], Alu.mult)
            nc.vector.tensor_scalar(m[:], pv_t[:], 0.2, None, Alu.is_lt)
            nc.vector.scalar_tensor_tensor(
                me[:], b2[:], -1.0, m[:], Alu.add, Alu.mult
            )
            nc.vector.scalar_tensor_tensor(
                diff[:], me[:], 1.0, b3[:], Alu.add, Alu.mult
            )
            nc.vector.tensor_tensor(term[:], diff[:], nlp[:], Alu.mult)
            osb = stp.tile([P, 1], f32)
            nc.vector.tensor_reduce(
                osb[:], term[:], axis=mybir.AxisListType.X, op=Alu.add
            )
            nc.sync.dma_start(out=out[:], in_=osb[:])

    nc.finalize()
    _built["fast"] = nc
    return nc


# ---------------------------------------------------------------------------
# Fallback: original all-f32 kernel (full-width scans, exact ACT exp).
# Correct for any input values / target distribution.
# ---------------------------------------------------------------------------

FB_CHUNKS = [1, 2, 5, 8, 8, 8, 8, 8, 8, 4, 2, 1, 1]
FB_WMAX = max(FB_CHUNKS) * C
FB_ACC = {3: 8, 7: 8, 8: 2, 9: 4, 10: 2}


def _build_fallback():
    if "fb" in _built:
        return _built["fb"]
    from concourse import bacc, mybir
    from concourse.tile import TileContext

    f32 = mybir.dt.float32
    f16 = mybir.dt.float16
    Alu = mybir.AluOpType
    Act = mybir.ActivationFunctionType

    nc = bacc.Bacc()
    x = nc.declare_dram_parameter("x", [NS, C], f32, isOutput=False)
    tgt = nc.declare_dram_parameter("tgt", [P, R], f32, isOutput=False)
    iot = nc.declare_dram_parameter("iota", [P, C], f32, isOutput=False)
    out = nc.declare_dram_parameter("out", [P, 1], f32, isOutput=True)

    with TileContext(nc) as tc:
        with (
            tc.tile_pool(name="const", bufs=1) as cpool,
            tc.tile_pool(name="xp", bufs=4) as xpool,
            tc.tile_pool(name="ep", bufs=2) as epool,
            tc.tile_pool(name="st", bufs=1) as stp,
        ):
            iota = cpool.tile([P, C], f32)
            nc.sync.dma_start(out=iota[:], in_=iot[:])
            tgt_sb = cpool.tile([P, R], f32)
            nc.sync.dma_start(out=tgt_sb[:], in_=tgt[:])

            s_all = stp.tile([P, R], f32)
            xt_all = stp.tile([P, R], f32)
            gsc_full = stp.tile([P, C], f32)

            ext = stp.tile([P, R], f32)
            lse = stp.tile([P, R], f32)
            rs = stp.tile([P, R], f32)
            pv = stp.tile([P, R], f32)
            nlp = stp.tile([P, R], f32)
            base = stp.tile([P, R], f32)
            b2 = stp.tile([P, R], f32)
            b3 = stp.tile([P, R], f32)
            m = stp.tile([P, R], f32)
            me = stp.tile([P, R], f32)
            diff = stp.tile([P, R], f32)
            term = stp.tile([P, R], f32)

            xv = x[:].rearrange("(p q) c -> p (q c)", p=P)
            col = 0
            for j, w in enumerate(FB_CHUNKS):
                xt_tile = xpool.tile([P, FB_WMAX], f32, tag="x")
                nc.sync.dma_start(
                    out=xt_tile[:, : w * C], in_=xv[:, col * C : (col + w) * C]
                )
                nacc = FB_ACC.get(j, 0)
                nbig = w - nacc
                esc = epool.tile([P, FB_WMAX], f16, tag="esc")
                if nbig:
                    nc.scalar.activation(
                        esc[:, : nbig * C], xt_tile[:, : nbig * C], Act.Exp
                    )
                for q in range(nbig, w):
                    cq = col + q
                    nc.scalar.activation(
                        esc[:, q * C : (q + 1) * C],
                        xt_tile[:, q * C : (q + 1) * C],
                        Act.Exp,
                        accum_out=s_all[:, cq : cq + 1],
                    )
                for q in range(w):
                    cq = col + q
                    nc.vector.scalar_tensor_tensor(
                        gsc_full[:],
                        iota[:],
                        tgt_sb[:, cq : cq + 1],
                        xt_tile[:, q * C : (q + 1) * C],
                        Alu.is_equal,
                        Alu.mult,
                        accum_out=xt_all[:, cq : cq + 1],
                    )
                if nbig:
                    nc.vector.tensor_reduce(
                        s_all[:, col : col + nbig],
                        esc[:, : nbig * C].rearrange("p (q c) -> p q c", q=nbig),
                        axis=mybir.AxisListType.X,
                        op=Alu.add,
                    )
                col += w

            nc.scalar.activation(ext[:], xt_all[:], Act.Exp)
            nc.scalar.activation(lse[:], s_all[:], Act.Ln)
            nc.vector.reciprocal(rs[:], s_all[:])
            nc.vector.tensor_tensor(pv[:], ext[:], rs[:], Alu.mult)
            nc.vector.tensor_tensor(nlp[:], lse[:], xt_all[:], Alu.subtract)
            nc.vector.tensor_scalar(
                base[:], pv[:], -2.0, 2.0, Alu.mult, Alu.add
            )
            nc.vector.tensor_tensor(b2[:], base[:], base[:], Alu.mult)
            nc.vector.tensor_tensor(b3[:], b2[:], base[:], Alu.mult)
            nc.vector.tensor_scalar(m[:], pv[:], 0.2, None, Alu.is_lt)
            nc.vector.scalar_tensor_tensor(
                me[:], b2[:], -1.0, m[:], Alu.add, Alu.mult
            )
            nc.vector.scalar_tensor_tensor(
                diff[:], me[:], 1.0, b3[:], Alu.add, Alu.mult
            )
            nc.vector.tensor_tensor(term[:], diff[:], nlp[:], Alu.mult)
            osb = stp.tile([P, 1], f32)
            nc.vector.tensor_reduce(
                osb[:], term[:], axis=mybir.AxisListType.X, op=Alu.add
            )
            nc.sync.dma_start(out=out[:], in_=osb[:])

    nc.finalize()
    _built["fb"] = nc
    return nc


def _build(full_scan=False):
    return _build_fallback() if full_scan else _build_fast()


def _prepare_in_maps(x, target):
    """Per core: sort rows by target, assign rank r -> (p=r%128, slot q=1+r//128)
    for r < 8064; top 128 ranks -> overflow slot q=0.  Build window masks and
    verify every slot's targets sit inside its static window."""
    x = np.asarray(x)
    if x.dtype != np.float32:
        x = x.astype(np.float32)
    t = np.asarray(target).astype(np.int64)
    fallback = bool(np.max(np.abs(x)) > X_ABS_LIMIT)
    cores = []
    for cid in range(NCORES):
        tc = t[cid * NS : (cid + 1) * NS]
        order = np.argsort(tc, kind="stable")
        ranks_main = order[: 128 * (R - 1)].reshape(R - 1, P)
        perm_pq = np.empty((P, R), dtype=np.int64)
        perm_pq[:, 1:] = ranks_main.T
        perm_pq[:, 0] = order[128 * (R - 1) :]
        perm = perm_pq.reshape(-1)
        tw = tc[perm_pq]  # [P, R] target at each slot
        wmask = np.zeros((P, R, STT_W), dtype=np.float16)
        col = 0
        for j, w in enumerate(CHUNKS):
            for k in range(w):
                q = col + k
                lo = CHUNK_LO[j] + CHUNK_STEP[j] * k
                pos = tw[:, q] - lo
                if not ((pos >= 0) & (pos < STT_W)).all():
                    fallback = True
                    continue
                wmask[np.arange(P), q, pos] = 1.0
            col += w
        cores.append((perm, perm_pq, wmask))
    np_dt = np.float32 if fallback else np.float16
    iota = np.ascontiguousarray(
        np.broadcast_to(np.arange(C, dtype=np_dt), (P, C))
    )
    in_maps = []
    for cid in range(NCORES):
        perm, perm_pq, wmask = cores[cid]
        xs = np.ascontiguousarray(x[cid * NS : (cid + 1) * NS][perm].astype(np_dt))
        tg = np.ascontiguousarray(
            t[cid * NS : (cid + 1) * NS][perm_pq].astype(np_dt)
        )
        if fallback:
            im = {"x": xs, "tgt": tg, "iota": iota}
        else:
            im = {"x": xs, "wmask": np.ascontiguousarray(wmask.reshape(P, R * STT_W))}
        in_maps.append(im)
    return in_maps, fallback


def _finish(results):
    total = 0.0
    for r in results:
        total += float(np.sum(r["out"].astype(np.float64)))
    return np.asarray(total, dtype=np.float32)


def kernel(x, target):
    from concourse.bass_utils import run_bass_kernel_spmd

    in_maps, fallback = _prepare_in_maps(x, target)
    nc = _build(fallback)
    res = run_bass_kernel_spmd(nc, in_maps, core_ids=list(range(NCORES)))
    return _finish(res.results)
